# revision 1
# baseline (speedup 1.0000x reference)
"""Trainium2 Bass kernel for nn_MoELayer (top-2 MoE, E=8 experts).

Strategy (expert-parallel across 8 NeuronCores):
  - Host computes the (tiny) gate matmul + top-2 + softmax, and dispatches
    each token to its two experts' cores ("all-to-all" done host-side as the
    sharding step). One expert per core.
  - Each core runs a Bass kernel computing, for its expert e and its routed
    tokens:   out = (silu(tok @ W1[e]) @ W2[e]) * gate_weight
    with bf16 matmul inputs and fp32 PSUM accumulation. Weights stay
    resident in SBUF; only the top-2-selected tokens are computed
    (4x fewer FLOPs than the dense reference).
  - Host scatter-adds the two weighted expert outputs per token.

Layouts (chosen so no on-device transposes are needed):
  stage 1:  actT[f, c] = silu( sum_d W1[d, f] * tokT[d, c] )
            matmul(lhsT=W1[dk, fj-tile], rhs=tokT[dk, c-chunk]) -> PSUM [f, c]
  stage 2:  out[c, d] = sum_f actT[f, c] * W2[f, d]
            matmul(lhsT=actT[fk, c-tile], rhs=W2[fk, d-chunk]) -> PSUM [c, d]

C (token capacity per core) is the exact max routed-token count, not
rounded up: stage 1 chunks may have non-multiple-of-128 widths and the
final stage-2 token tile may have <128 partitions.
"""

import math
import sys

sys.path.insert(0, "/opt/trn_rl_repo")

import ml_dtypes
import numpy as np

B, T, D, F, E = 2, 2048, 1024, 4096, 8
N = B * T
P = 128
KD = D // P  # 8
KF = F // P  # 32

bf16 = ml_dtypes.bfloat16

_nc_cache: dict[int, object] = {}
LAST_RESULTS = None  # BassKernelResults from the most recent run (for test.py)
TRACE = False


def _chunk_sizes(C: int) -> list[int]:
    """Split C into near-equal chunks of <=512 (stage-1 matmul free dim /
    PSUM bank limit), smallest first so the critical first token transfer
    is as small as possible."""
    n = math.ceil(C / 512)
    base = math.ceil(C / (n * P)) * P
    sizes = []
    rem = C
    while rem > 0:
        s = min(base, rem)
        sizes.append(s)
        rem -= s
    return sorted(sizes)


def _build(C: int):
    import concourse.mybir as mybir
    import concourse.tile as tile
    from concourse import bacc

    dt = mybir.dt

    nc = bacc.Bacc(None, target_bir_lowering=False)

    chunks = _chunk_sizes(C)

    # one token tensor per chunk -> fully contiguous per-partition DMA
    # packets (KD*cn*2 bytes) instead of 768B strided slices
    tokts = [
        nc.dram_tensor(f"tokt{i}", [P, KD, cn], dt.bfloat16, kind="ExternalInput")
        for i, cn in enumerate(chunks)
    ]
    w1 = nc.dram_tensor("w1", [P, KD, F], dt.bfloat16, kind="ExternalInput")
    w2 = nc.dram_tensor("w2", [P, KF, D], dt.bfloat16, kind="ExternalInput")
    # output is transposed: [D, C] with D on partitions; the gate-weight
    # scale + transpose happen on the host during scatter-add
    out = nc.dram_tensor("out", [D, C], dt.float32, kind="ExternalOutput")

    with tile.TileContext(nc) as tc:
        with (
            tc.tile_pool(name="const", bufs=1) as cpool,
            tc.tile_pool(name="act", bufs=1) as apool,
            tc.tile_pool(name="ps1", bufs=2, space="PSUM") as ps1pool,
            tc.tile_pool(name="ps2", bufs=2, space="PSUM") as ps2pool,
            tc.tile_pool(name="ob", bufs=4) as opool,
        ):
            w1_sb = cpool.tile([P, KD, F], dt.bfloat16, tag="w1")
            w2_sb = cpool.tile([P, KF, D], dt.bfloat16, tag="w2")
            tok_sbs = [
                cpool.tile(
                    [P, KD, cn], dt.bfloat16, tag=f"tok{i}", name=f"tok_sb{i}"
                )
                for i, cn in enumerate(chunks)
            ]

            # Input loads, all on the sync engine's HW DGE (SW DGE via other
            # engines measured far slower), emission-ordered by first use:
            # chunk-0 tokens, W1 quarters, remaining tokens, then W2.
            nc.sync.dma_start(tok_sbs[0][:], tokts[0][:])
            FQ = F // 4
            for q in range(4):
                nc.sync.dma_start(
                    w1_sb[:, :, q * FQ : (q + 1) * FQ],
                    w1[:, :, q * FQ : (q + 1) * FQ],
                )
            for i in range(1, len(chunks)):
                nc.sync.dma_start(tok_sbs[i][:], tokts[i][:])
            for q in range(4):
                nc.sync.dma_start(
                    w2_sb[:, q * (KF // 4) : (q + 1) * (KF // 4), :],
                    w2[:, q * (KF // 4) : (q + 1) * (KF // 4), :],
                )

            c0 = 0
            for ci, cn in enumerate(chunks):
                tok_sb = tok_sbs[ci]
                act_sb = apool.tile([P, KF, cn], dt.bfloat16, tag="act")
                # ---- stage 1: actT = silu(W1^T @ tokT) ----
                for fj in range(KF):
                    ps1 = ps1pool.tile([P, cn], dt.float32, tag="ps1")
                    for dk in range(KD):
                        nc.tensor.matmul(
                            ps1[:],
                            w1_sb[:, dk, fj * P : (fj + 1) * P],
                            tok_sb[:, dk, :],
                            start=(dk == 0),
                            stop=(dk == KD - 1),
                        )
                    nc.scalar.activation(
                        act_sb[:, fj, :],
                        ps1[:],
                        mybir.ActivationFunctionType.Silu,
                    )
                # ---- stage 2: outT = W2^T @ actT  (D on partitions,
                # tokens on the free dim -> no padded token tiles) ----
                for dm in range(D // P):
                    ps2 = ps2pool.tile([P, cn], dt.float32, tag="ps2")
                    for fk in range(KF):
                        nc.tensor.matmul(
                            ps2[:],
                            w2_sb[:, fk, dm * P : (dm + 1) * P],
                            act_sb[:, fk, :],
                            start=(fk == 0),
                            stop=(fk == KF - 1),
                        )
                    ob = opool.tile([P, cn], dt.float32, tag="ob")
                    nc.vector.tensor_copy(ob[:], ps2[:])
                    nc.sync.dma_start(
                        out[dm * P : (dm + 1) * P, c0 : c0 + cn],
                        ob[:],
                    )
                c0 += cn

    nc.compile()
    return nc


def _get_nc(C: int):
    if C not in _nc_cache:
        _nc_cache[C] = _build(C)
    return _nc_cache[C]


def kernel(**inputs) -> np.ndarray:
    global LAST_RESULTS
    x = np.asarray(inputs["x"], dtype=np.float32)
    Wg = np.asarray(inputs["Wg"], dtype=np.float32)
    W1 = np.asarray(inputs["W1"], dtype=np.float32)
    W2 = np.asarray(inputs["W2"], dtype=np.float32)

    h = np.ascontiguousarray(x.reshape(N, D))

    # ---- host gate: top-2 + softmax (0.05% of total FLOPs) ----
    logits = h @ Wg.T  # [N, E] f32
    idx2 = np.argpartition(-logits, 1, axis=1)[:, :2]
    lsel = np.take_along_axis(logits, idx2, axis=1)
    first = lsel[:, 0] >= lsel[:, 1]
    i0 = np.where(first, idx2[:, 0], idx2[:, 1])
    i1 = np.where(first, idx2[:, 1], idx2[:, 0])
    l0 = np.where(first, lsel[:, 0], lsel[:, 1])
    l1 = np.where(first, lsel[:, 1], lsel[:, 0])
    e1 = np.exp((l1 - l0).astype(np.float32))
    w0 = (1.0 / (1.0 + e1)).astype(np.float32)
    w1g = (e1 / (1.0 + e1)).astype(np.float32)

    token_ids = np.concatenate([np.arange(N), np.arange(N)])
    expert_ids = np.concatenate([i0, i1])
    gate_w = np.concatenate([w0, w1g])

    counts = np.bincount(expert_ids, minlength=E)
    C = int(counts.max())

    hb = h.astype(bf16)
    W1b = W1.astype(bf16)
    W2b = W2.astype(bf16)

    in_maps = []
    ids_per_expert = []
    gw_per_expert = []
    for e in range(E):
        sel = np.flatnonzero(expert_ids == e)
        ids_e = token_ids[sel]
        n_e = len(ids_e)
        ids_per_expert.append(ids_e)
        gw_per_expert.append(gate_w[sel])

        tokT = np.zeros((P, KD, C), dtype=bf16)
        # tokens [n,D] -> [D,n] -> [KD,P,n] -> [P,KD,n]
        tokT[:, :, :n_e] = (
            hb[ids_e].T.reshape(KD, P, n_e).transpose(1, 0, 2)
        )
        m = {
            "w1": np.ascontiguousarray(
                W1b[e].reshape(KD, P, F).transpose(1, 0, 2)
            ),
            "w2": np.ascontiguousarray(
                W2b[e].reshape(KF, P, D).transpose(1, 0, 2)
            ),
        }
        c0 = 0
        for i, cn in enumerate(_chunk_sizes(C)):
            m[f"tokt{i}"] = np.ascontiguousarray(tokT[:, :, c0 : c0 + cn])
            c0 += cn
        in_maps.append(m)

    nc = _get_nc(C)
    from concourse.bass_utils import run_bass_kernel_spmd

    LAST_RESULTS = run_bass_kernel_spmd(
        nc, in_maps, core_ids=list(range(E)), trace=TRACE
    )

    y = np.zeros((N, D), dtype=np.float32)
    for e in range(E):
        o = np.asarray(LAST_RESULTS.results[e]["out"], dtype=np.float32)  # [D, C]
        ids_e = ids_per_expert[e]
        n_e = len(ids_e)
        y[ids_e] += gw_per_expert[e][:, None] * o[:, :n_e].T
    return y.reshape(B, T, D)



# revision 2
# speedup vs baseline: 1.2286x; 1.2286x over previous
"""Trainium2 Bass kernel for nn_MoELayer (top-2 MoE, E=8 experts).

Strategy (expert-parallel across 8 NeuronCores):
  - Host computes the (tiny) gate matmul + top-2 + softmax and dispatches
    each token to its two experts' cores. One expert per core.
  - Token capacity per core is fixed at C=1024 (2 PSUM-friendly chunks of
    512). The few overflow tokens of experts routed >1024 tokens (~1% of
    FLOPs) are computed on the host in fp32 and folded into the
    scatter-add, keeping every core's device work identical and minimal.
  - Each core runs a RAW-BASS (hand-synchronized) kernel computing, for
    its expert e and its <=1024 routed tokens:
        out = silu(tok @ W1[e]) @ W2[e]
    with bf16 matmul inputs and fp32 PSUM accumulation.

Why raw bass instead of Tile: Tile attaches a semaphore increment to
EVERY matmul (each accumulation-group member has the PSUM-reading
activation as a descendant).  A PE-engine semaphore update costs ~31ns
on the engine timeline (SEM_PROP_BASE 17 + PE send overhead 14), which
measured as a ~33ns/MM tax (~48us total) in the Tile baseline.  Here
semaphores are placed by hand: one inc per 8-matmul accumulation group.

Layouts (no on-device transposes):
  stage 1:  actT[f, c] = silu( sum_d W1[d, f] * tokT[d, c] )
  stage 2:  outT[d, c] = sum_f W2[f, d] * actT[f, c]

DMA plan: W1 streams on the scalar-engine HWDGE ring in 4 fj-ordered
slabs (so the first matmul only waits for tok chunk 0 + slab 0), while
tokens/W2 stream on the sync-engine ring.  Every input DMA is a single
contiguous-per-partition block (own DRAM tensor per slab) for >=4KB
descriptors (full DMA line rate).  ~55 throwaway warm-up matmuls run
during the initial DMA wait to lift the PE HAM clock-gate to 2.4GHz
before real work starts.
"""

import math
import os
import sys

sys.path.insert(0, "/opt/trn_rl_repo")

import ml_dtypes
import numpy as np

B, T, D, F, E = 2, 2048, 1024, 4096, 8
N = B * T
P = 128
KD = D // P  # 8
KF = F // P  # 32

C = 1024  # device token capacity per expert/core
CH = 512  # chunk width (PSUM bank = 512 fp32)
NCHUNK = C // CH  # 2

# W1 streamed in fj-ordered F-col slabs; first slab small for fast start.
W1_SLABS = [512, 1024, 1024, 1536]
W1_BASES = [0, 512, 1536, 2560]
W2_SLAB = 256  # D-cols per W2 slab (4 slabs)
N_WARM = int(os.environ.get("N_WARM", "55"))

bf16 = ml_dtypes.bfloat16

_nc_cache: dict = {}
LAST_RESULTS = None  # BassKernelResults from the most recent run (for test.py)
TRACE = False


def _w1_slab_of(fj: int) -> int:
    c0 = fj * P
    for i in range(len(W1_SLABS) - 1, -1, -1):
        if c0 >= W1_BASES[i]:
            return i
    raise AssertionError


def _build():
    import concourse.mybir as mybir
    from concourse import bacc

    dt = mybir.dt
    nc = bacc.Bacc(None, target_bir_lowering=False)

    # ---- DRAM tensors (each a contiguous block => big DMA descriptors) ----
    toks = [
        nc.dram_tensor(f"tok{c}", [P, KD, CH], dt.bfloat16, kind="ExternalInput")
        for c in range(NCHUNK)
    ]
    w1s = [
        nc.dram_tensor(f"w1s{i}", [P, KD, s], dt.bfloat16, kind="ExternalInput")
        for i, s in enumerate(W1_SLABS)
    ]
    w2s = [
        nc.dram_tensor(f"w2s{i}", [P, KF, W2_SLAB], dt.bfloat16, kind="ExternalInput")
        for i in range(D // W2_SLAB)
    ]
    out = nc.dram_tensor("out", [D, C], dt.float32, kind="ExternalOutput")

    from contextlib import ExitStack

    stack = ExitStack()
    sb = lambda name, shape, dty: stack.enter_context(nc.sbuf_tensor(name, shape, dty))
    ps = lambda name, shape: stack.enter_context(
        nc.psum_tensor(name, shape, dt.float32)
    )
    sem = lambda name: stack.enter_context(nc.semaphore(name))

    tok_sb = [sb(f"tok_sb{c}", [P, KD, CH], dt.bfloat16) for c in range(NCHUNK)]
    w1_sb = [sb(f"w1_sb{i}", [P, KD, s], dt.bfloat16) for i, s in enumerate(W1_SLABS)]
    w2_sb = [sb(f"w2_sb{i}", [P, KF, W2_SLAB], dt.bfloat16) for i in range(len(w2s))]
    act_sb = sb("act_sb", [P, KF, CH], dt.bfloat16)
    ob_sb = sb("ob_sb", [P, 4, CH], dt.float32)  # 4 rotating output buffers
    warm_sb = sb("warm_sb", [P, P], dt.bfloat16)

    ps1 = [ps(f"ps1_{b}", [P, CH]) for b in range(4)]  # stage-1 banks
    ps2 = [ps(f"ps2_{b}", [P, CH]) for b in range(2)]  # stage-2 banks
    warm_ps = ps("warm_ps", [P, CH])

    in_sem = sem("in_sem")  # sync-ring input DMAs
    w1_sem = sem("w1_sem")  # scalar-ring W1 DMAs
    pe_sem = sem("pe_sem")  # PE accumulation-group completions
    sc_sem = sem("sc_sem")  # scalar silu completions
    vec_sem = sem("vec_sem")  # vector psum->sbuf copy completions
    od_sem = sem("od_sem")  # output DMA completions
    ms_sem = sem("ms_sem")  # warm-tile memset

    silu = mybir.ActivationFunctionType.Silu

    # =================== sync engine (SP HWDGE ring) ===================
    # inputs: tok0 ->16, w2s0..3 ->32,48,64,80, tok1 ->96
    nc.sync.dma_start(tok_sb[0][:], toks[0][:]).then_inc(in_sem, 16)
    for i in range(len(w2s)):
        nc.sync.dma_start(w2_sb[i][:], w2s[i][:]).then_inc(in_sem, 16)
    nc.sync.dma_start(tok_sb[1][:], toks[1][:]).then_inc(in_sem, 16)
    TOK_THRESH = [16, 16 + 16 * len(w2s) + 16]  # tok chunk ready thresholds
    W2_THRESH = lambda s: 32 + 16 * s  # w2 slab s ready

    # output DMAs (emitted now; they execute in order, each gated on its copy)
    for c in range(NCHUNK):
        for dm in range(KD):
            g2 = c * KD + dm
            nc.sync.wait_ge(vec_sem, g2 + 1)
            nc.sync.dma_start(
                out[dm * P : (dm + 1) * P, c * CH : (c + 1) * CH],
                ob_sb[:, g2 % 4, :],
            ).then_inc(od_sem, 16)
    nc.sync.wait_ge(od_sem, 16 * NCHUNK * KD)

    # =================== scalar engine (ACT HWDGE ring) ===================
    for i in range(len(w1s)):
        nc.scalar.dma_start(w1_sb[i][:], w1s[i][:]).then_inc(w1_sem, 16)

    for c in range(NCHUNK):
        for fj in range(KF):
            s_glob = c * KF + fj
            nc.scalar.wait_ge(pe_sem, c * 40 + fj + 1)
            nc.scalar.activation(
                act_sb[:, fj, :], ps1[s_glob % 4][:], silu
            ).then_inc(sc_sem, 1)

    # =================== gpsimd: init warm tile ===================
    nc.gpsimd.memset(warm_sb[:], 0.0).then_inc(ms_sem, 1)

    # =================== tensor engine ===================
    # warm-up: keep PE busy during the input-DMA wait so the HAM clock
    # gate reaches 8/8 before real matmuls start.
    nc.tensor.wait_ge(ms_sem, 1)
    for _ in range(N_WARM):
        nc.tensor.matmul(warm_ps[:, 0:P], warm_sb[:], warm_sb[:], start=True, stop=True)

    for c in range(NCHUNK):
        # ---- stage 1: act[f, :] = silu(sum_d w1[d, f] * tok[d, :]) ----
        for fj in range(KF):
            s_glob = c * KF + fj
            sl = _w1_slab_of(fj)
            col = fj * P - W1_BASES[sl]
            if fj == 0:
                nc.tensor.wait_ge(in_sem, TOK_THRESH[c])
            if fj == 0 or _w1_slab_of(fj - 1) != sl:
                nc.tensor.wait_ge(w1_sem, 16 * (sl + 1))
            if s_glob >= 4:  # ps1 bank WAR vs silu of group s_glob-4
                nc.tensor.wait_ge(sc_sem, s_glob - 3)
            for dk in range(KD):
                mm = nc.tensor.matmul(
                    ps1[s_glob % 4][:],
                    w1_sb[sl][:, dk, col : col + P],
                    tok_sb[c][:, dk, :],
                    start=(dk == 0),
                    stop=(dk == KD - 1),
                )
            mm.then_inc(pe_sem, 1)
        # ---- stage 2: out[d, :] = sum_f w2[f, d] * act[f, :] ----
        for dm in range(KD):
            g2 = c * KD + dm
            if dm % 2 == 0:
                nc.tensor.wait_ge(in_sem, W2_THRESH(dm // 2))
            if g2 >= 2:  # ps2 bank WAR vs copy of group g2-2
                nc.tensor.wait_ge(vec_sem, g2 - 1)
            for fk in range(KF):
                if dm == 0:  # act readiness, per-MM so there is no stall
                    nc.tensor.wait_ge(sc_sem, c * KF + fk + 1)
                mm = nc.tensor.matmul(
                    ps2[g2 % 2][:],
                    w2_sb[dm // 2][:, fk, (dm % 2) * P : (dm % 2) * P + P],
                    act_sb[:, fk, :],
                    start=(fk == 0),
                    stop=(fk == KF - 1),
                )
            mm.then_inc(pe_sem, 1)

    # =================== vector engine ===================
    for c in range(NCHUNK):
        for dm in range(KD):
            g2 = c * KD + dm
            nc.vector.wait_ge(pe_sem, c * 40 + KF + dm + 1)
            if g2 >= 4:  # ob buffer WAR vs out-DMA of copy g2-4
                nc.vector.wait_ge(od_sem, 16 * (g2 - 3))
            nc.vector.tensor_copy(ob_sb[:, g2 % 4, :], ps2[g2 % 2][:]).then_inc(
                vec_sem, 1
            )

    nc.compile()
    stack.close()
    return nc


def _get_nc():
    if "nc" not in _nc_cache:
        _nc_cache["nc"] = _build()
    return _nc_cache["nc"]


def kernel(**inputs) -> np.ndarray:
    global LAST_RESULTS
    x = np.asarray(inputs["x"], dtype=np.float32)
    Wg = np.asarray(inputs["Wg"], dtype=np.float32)
    W1 = np.asarray(inputs["W1"], dtype=np.float32)
    W2 = np.asarray(inputs["W2"], dtype=np.float32)

    h = np.ascontiguousarray(x.reshape(N, D))

    # ---- host gate: top-2 + softmax (0.05% of total FLOPs) ----
    logits = h @ Wg.T  # [N, E] f32
    idx2 = np.argpartition(-logits, 1, axis=1)[:, :2]
    lsel = np.take_along_axis(logits, idx2, axis=1)
    first = lsel[:, 0] >= lsel[:, 1]
    i0 = np.where(first, idx2[:, 0], idx2[:, 1])
    i1 = np.where(first, idx2[:, 1], idx2[:, 0])
    l0 = np.where(first, lsel[:, 0], lsel[:, 1])
    l1 = np.where(first, lsel[:, 1], lsel[:, 0])
    e1 = np.exp((l1 - l0).astype(np.float32))
    w0 = (1.0 / (1.0 + e1)).astype(np.float32)
    w1g = (e1 / (1.0 + e1)).astype(np.float32)

    token_ids = np.concatenate([np.arange(N), np.arange(N)])
    expert_ids = np.concatenate([i0, i1])
    gate_w = np.concatenate([w0, w1g])

    hb = h.astype(bf16)
    W1b = W1.astype(bf16)
    W2b = W2.astype(bf16)

    y = np.zeros((N, D), dtype=np.float32)

    in_maps = []
    ids_per_expert = []
    gw_per_expert = []
    for e in range(E):
        sel = np.flatnonzero(expert_ids == e)
        ids_e = token_ids[sel]
        gw_e = gate_w[sel]
        if len(ids_e) > C:
            # capacity overflow -> host fp32 FFN, folded into scatter-add
            ov_ids = ids_e[C:]
            ov_gw = gw_e[C:]
            up = h[ov_ids] @ W1[e]
            act = up * (1.0 / (1.0 + np.exp(-up)))
            y[ov_ids] += ov_gw[:, None] * (act @ W2[e])
            ids_e = ids_e[:C]
            gw_e = gw_e[:C]
        n_e = len(ids_e)
        ids_per_expert.append(ids_e)
        gw_per_expert.append(gw_e)

        tokT = np.zeros((P, KD, C), dtype=bf16)
        # tokens [n,D] -> [D,n] -> [KD,P,n] -> [P,KD,n]
        tokT[:, :, :n_e] = hb[ids_e].T.reshape(KD, P, n_e).transpose(1, 0, 2)
        W1pe = W1b[e].reshape(KD, P, F).transpose(1, 0, 2)  # [P, KD, F]
        W2pe = W2b[e].reshape(KF, P, D).transpose(1, 0, 2)  # [P, KF, D]
        m = {}
        for c in range(NCHUNK):
            m[f"tok{c}"] = np.ascontiguousarray(tokT[:, :, c * CH : (c + 1) * CH])
        for i, s in enumerate(W1_SLABS):
            m[f"w1s{i}"] = np.ascontiguousarray(
                W1pe[:, :, W1_BASES[i] : W1_BASES[i] + s]
            )
        for i in range(D // W2_SLAB):
            m[f"w2s{i}"] = np.ascontiguousarray(
                W2pe[:, :, i * W2_SLAB : (i + 1) * W2_SLAB]
            )
        in_maps.append(m)

    nc = _get_nc()
    from concourse.bass_utils import run_bass_kernel_spmd

    LAST_RESULTS = run_bass_kernel_spmd(
        nc, in_maps, core_ids=list(range(E)), trace=TRACE
    )

    for e in range(E):
        o = np.asarray(LAST_RESULTS.results[e]["out"], dtype=np.float32)  # [D, C]
        ids_e = ids_per_expert[e]
        n_e = len(ids_e)
        y[ids_e] += gw_per_expert[e][:, None] * o[:, :n_e].T
    return y.reshape(B, T, D)
